# revision 5
# baseline (speedup 1.0000x reference)
"""Trainium2 Bass kernel for nn_Loss_2482491097912 (SimCLR-style semi-supervised loss).

Strategy (8 NeuronCores, data-parallel over anchor rows):
  - Each core receives the FULL z1/z2 (pre-transposed + bf16 on host) and its
    1024-row slice of the masks, with h-columns ROLLED so that every core's
    local rows sit at columns [0:1024] -> one shared SPMD program.
  - On-core: projection MLP in transposed layout (HT = [feat, rows]),
    column-normalize (1/||h|| via Ln/Exp), then sim matmuls + exp with fused
    row-sum accumulation, masked sums via DVE tensor ops.
  - v2 improvements over the first working version:
      * S21 is never computed: its row sums are column sums of E12 (PSUM-
        accumulated ones-matmuls) and its masked sums use host-transposed
        masks applied to E12.
      * S11/S22 exploit symmetry: only 5 of 8 rolled column blocks are
        computed per core (cyclic triangle cover); the missing row-sum
        pieces come from column sums of blocks 1..3.
      * exp output and masked-sum operands are bf16 (2x DVE throughput);
        masks are shipped as bf16 from host.
      * activation-table steering: all scalar-engine functions resolve to
        the one table set that holds Exp+Ln+Square, removing ~65 table
        loads (~2.7us each) from the critical path.
  - Host combines per-core row sums / col sums / masked sums / diag dots
    into the two scalar losses (cheap numpy tail math).
"""

import numpy as np
import ml_dtypes

import concourse.bass as bass
import concourse.bacc as bacc
import concourse.tile as tile
import concourse.mybir as mybir
from concourse.bass_utils import run_bass_kernel_spmd

F32 = mybir.dt.float32
BF16 = mybir.dt.bfloat16
AF = mybir.ActivationFunctionType
OP = mybir.AluOpType

N = 8192
D = 512
NCORES = 8
RPC = N // NCORES          # rows per core = 1024
NBLK = RPC // 128          # row blocks per core = 8
NH = 2 * N                 # 16384 stacked rows (h1 then h2)
PCH = 512                  # projection column chunk
NPCH = NH // PCH           # 32 projection chunks
CCH = 1024                 # phase-2 sim column chunk
NC12 = N // CCH            # 8 column chunks for S12
NCTRI = 5                  # 5 rolled column blocks for S11/S22 (triangle)
CSUM_TRI = (1, 2, 3)       # triangle chunks whose col sums we need

# acc_all rowsum/masked-sum column map (one f32 col per (sim, chunk, block)).
COL_S12 = 0                            # 8c x 8b = 64
COL_S11 = 64                           # 5c x 8b = 40
COL_S22 = 104                          # 40
COL_P12 = 144                          # 64  (E12 * pos)
COL_N12 = 208                          # 64  (E12 * neg)
COL_P21 = 272                          # 64  (E12 * pos^T)
COL_N21 = 336                          # 64  (E12 * neg^T)
ACC_COLS = 400

# csum output layout: [1, 14336] f32
CS_S12 = 0                 # 8192: colsums of E12 (rolled cols)
CS_S11 = N                 # 3072: colsums of E11 rolled chunks 1..3
CS_S22 = N + 3072          # 3072
CSUM_COLS = N + 6144


def _patch_act_tables():
    """Steer the act-table-load pass to the one set that holds Exp+Ln+Square
    (+Identity/Copy) so phase 1 never switches table sets."""
    import functools
    import concourse.hw_specs as hw_specs

    if getattr(bacc.get_activation_tables, "_steered", False):
        return
    orig = hw_specs.get_activation_tables
    keep_funcs = {AF.Exp, AF.Ln, AF.Square, AF.Identity, AF.Copy}

    @functools.cache
    def steered(arch):
        tabs = orig(arch)
        best = None
        for name, s in tabs.items():
            if keep_funcs <= s:
                best = name
                break
        if best is None:
            return tabs
        return {
            name: (s if name == best else (s - keep_funcs))
            for name, s in tabs.items()
        }

    steered._steered = True
    bacc.get_activation_tables = steered
    hw_specs.get_activation_tables = steered


def _emit(nc, tc, reps=1):
    zt = nc.dram_tensor("zt", [D, NH], BF16, kind="ExternalInput").ap()
    w1t = nc.dram_tensor("w1t", [D + 1, D], BF16, kind="ExternalInput").ap()
    w2t = nc.dram_tensor("w2t", [D, D], BF16, kind="ExternalInput").ap()
    b2t = nc.dram_tensor("b2t", [D, 1], F32, kind="ExternalInput").ap()
    pm = nc.dram_tensor("pm", [RPC, N], BF16, kind="ExternalInput").ap()
    nm = nc.dram_tensor("nm", [RPC, N], BF16, kind="ExternalInput").ap()
    pmt = nc.dram_tensor("pmt", [RPC, N], BF16, kind="ExternalInput").ap()
    nmt = nc.dram_tensor("nmt", [RPC, N], BF16, kind="ExternalInput").ap()

    acc_out = nc.dram_tensor("acc", [128, ACC_COLS], F32, kind="ExternalOutput").ap()
    csum_out = nc.dram_tensor("csum", [1, CSUM_COLS], F32, kind="ExternalOutput").ap()
    d12_out = nc.dram_tensor("d12p", [1, RPC], F32, kind="ExternalOutput").ap()
    for _ in range(reps):
        _emit_body(nc, tc, zt, w1t, w2t, b2t, pm, nm, pmt, nmt,
                   acc_out, csum_out, d12_out)


def _emit_body(nc, tc, zt, w1t, w2t, b2t, pm, nm, pmt, nmt,
               acc_out, csum_out, d12_out):

    zt_r = zt.rearrange("(t p) n -> p t n", p=128)      # [128, 4, 16384]
    w1t_r = w1t[0:D, :].rearrange("(t p) m -> p t m", p=128)
    w2t_r = w2t.rearrange("(t p) m -> p t m", p=128)
    b2t_r = b2t.rearrange("(t p) 1 -> p t", p=128)      # [128, 4]

    top = tc.alloc_tile_pool(name="top", bufs=1)
    # resident normalized H^T, 4 k-tiles of [128, 16384] bf16 (32KB/part each)
    ht = [top.tile([128, NH], BF16, name=f"ht{k}", tag=f"ht{k}") for k in range(4)]
    acc_all = top.tile([128, ACC_COLS], F32, name="acc_all")
    w1_sb = top.tile([128, 4, D], BF16, name="w1_sb")
    b1_sb = top.tile([1, D], BF16, name="b1_sb")
    w2_sb = top.tile([128, 4, D], BF16, name="w2_sb")
    b2_sb = top.tile([128, 4], F32, name="b2_sb")
    ones_r = top.tile([1, D], BF16, name="ones_r")      # rhs for L1 bias matmul
    ones_cb = top.tile([128, 1], BF16, name="ones_cb")  # bf16 lhsT for col sums

    nc.sync.dma_start(w1_sb[:], w1t_r)
    nc.sync.dma_start(b1_sb[:], w1t[D:D + 1, :])
    nc.sync.dma_start(w2_sb[:], w2t_r)
    nc.sync.dma_start(b2_sb[:], b2t_r)
    nc.vector.memset(ones_r[:], 1.0)
    nc.vector.memset(ones_cb[:], 1.0)
    nc.vector.memset(acc_all[:], 0.0)

    # ---------------- Phase 1: projection (transposed layout) ----------------
    with (
        tc.tile_pool(name="pj_sb", bufs=2) as pj,
        tc.tile_pool(name="pp_l1", bufs=2, space="PSUM") as pp_l1,
        tc.tile_pool(name="pp_l2", bufs=1, space="PSUM") as pp_l2,
        tc.tile_pool(name="pp_n", bufs=2, space="PSUM") as pp_n,
    ):
        for c in range(NPCH):
            cs = c * PCH
            zt_t = pj.tile([128, 4, PCH], BF16, name="zt_t", tag="zt")
            nc.sync.dma_start(zt_t[:], zt_r[:, :, cs:cs + PCH])

            gts = []
            for m in range(4):
                ms = m * 128
                l1_ps = pp_l1.tile([128, PCH], F32, name="l1_ps", tag="l1")
                for k in range(4):
                    nc.tensor.matmul(
                        l1_ps[:], w1_sb[:, k, ms:ms + 128], zt_t[:, k, :],
                        start=(k == 0), stop=False)
                # bias row via K=1 matmul: adds b1[m-chunk] to all columns
                nc.tensor.matmul(
                    l1_ps[:], b1_sb[:, ms:ms + 128], ones_r[:, 0:PCH],
                    start=False, stop=True)
                t_sb = pj.tile([128, PCH], F32, name="t_sb", tag="texp")
                nc.scalar.activation(t_sb[:], l1_ps[:], AF.Exp)
                gt = pj.tile([128, PCH], BF16, name="gt", tag=f"gt{m}")
                # elu(u) = min(exp(u) - 1, u)
                nc.vector.scalar_tensor_tensor(
                    gt[:], t_sb[:], 1.0, l1_ps[:], op0=OP.subtract, op1=OP.min)
                gts.append(gt)

            norms_ps = pp_n.tile([1, PCH], F32, name="norms_ps", tag="n")
            l2s = []
            for m in range(4):
                ms = m * 128
                l2_ps = pp_l2.tile([128, PCH], F32, name="l2_ps", tag=f"l2{m}")
                for k in range(4):
                    nc.tensor.matmul(
                        l2_ps[:], w2_sb[:, k, ms:ms + 128], gts[k][:],
                        start=(k == 0), stop=(k == 3))
                sq_sb = pj.tile([128, PCH], BF16, name="sq_sb", tag="sq")
                nc.scalar.activation(
                    sq_sb[:], l2_ps[:], AF.Square, bias=b2_sb[:, m:m + 1])
                nc.tensor.matmul(
                    norms_ps[:], ones_cb[:], sq_sb[:],
                    start=(m == 0), stop=(m == 3))
                l2s.append(l2_ps)

            # r = 1/sqrt(norms^2) = exp(-0.5 * ln(norms^2)), then broadcast
            lg = pj.tile([1, PCH], F32, name="lg", tag="lg")
            nc.scalar.activation(lg[:], norms_ps[:], AF.Ln)
            r1 = pj.tile([1, PCH], F32, name="r1", tag="r1")
            nc.scalar.activation(r1[:], lg[:], AF.Exp, scale=-0.5)
            rb = pj.tile([128, PCH], F32, name="rb", tag="rb")
            nc.gpsimd.partition_broadcast(rb[:], r1[:])

            for m in range(4):
                # htn = (h + b2) * r  -> bf16 into resident HT
                nc.vector.scalar_tensor_tensor(
                    ht[m][:, cs:cs + PCH], l2s[m][:], b2_sb[:, m:m + 1], rb[:],
                    op0=OP.add, op1=OP.mult)

    # ---------------- Phase 2 pools (masks prefetch before d12) --------------
    mk = tc.alloc_tile_pool(name="mk_sb", bufs=2)
    MASK_SRCS = (pm, nm, pmt, nmt)

    def mask_tiles(c, b):
        tiles = []
        for mi, src in enumerate(MASK_SRCS):
            t = mk.tile([128, CCH], BF16, name=f"m{mi}", tag=f"m{mi}")
            nc.sync.dma_start(
                t[:], src[b * 128:(b + 1) * 128, c * CCH:(c + 1) * CCH])
            tiles.append(t)
        return tiles

    # Build the interleaved unit schedule: S12 units carry the DVE load
    # (4 masked stt each), triangle units are PE-only; interleave so both
    # engines stay busy.
    s12_units = [("S12", c, b) for c in range(NC12) for b in range(NBLK)]
    tri_units = ([("S11", c, b) for c in range(NCTRI) for b in range(NBLK)] +
                 [("S22", c, b) for c in range(NCTRI) for b in range(NBLK)])
    tagged = ([((i + 0.5) / len(s12_units), u) for i, u in enumerate(s12_units)] +
              [((i + 0.5) / len(tri_units), u) for i, u in enumerate(tri_units)])
    tagged.sort(key=lambda t: t[0])
    units = [u for _, u in tagged]

    # masks are prefetched one full S12 unit ahead; seed the pipeline here so
    # the first DMA overlaps the d12 phase
    s12_next = {s12_units[i]: s12_units[i + 1]
                for i in range(len(s12_units) - 1)}
    masks_for = {s12_units[0]: mask_tiles(s12_units[0][1], s12_units[0][2])}

    # ------------- Phase 1.5: d12 = rowwise dot n1.n2 for local rows ---------
    with (
        tc.tile_pool(name="dd_sb", bufs=2) as dd,
        tc.tile_pool(name="dd_ps", bufs=2, space="PSUM") as dd_ps,
    ):
        d12_sb = dd.tile([1, RPC], F32, name="d12_sb", bufs=1)
        for h in range(2):
            hs = h * 512
            dps = dd_ps.tile([1, 512], F32, name="dps", tag="dps")
            for k in range(4):
                mt = dd.tile([128, 512], BF16, name="mt", tag="mt")
                nc.vector.tensor_mul(
                    mt[:], ht[k][:, hs:hs + 512], ht[k][:, N + hs:N + hs + 512])
                nc.tensor.matmul(dps[:], ones_cb[:], mt[:],
                                 start=(k == 0), stop=(k == 3))
            nc.scalar.copy(d12_sb[:, hs:hs + 512], dps[:])
        nc.sync.dma_start(d12_out[:], d12_sb[:])

    # ---------------- Phase 2: sims + exp row-sums + masked/col sums ---------
    SIM_CFG = {
        # lhs half, rhs half, rowsum col base, csum chunks, csum col base
        "S12": (0, 1, COL_S12, set(range(NC12)), CS_S12, 0),
        "S11": (0, 0, COL_S11, set(CSUM_TRI), CS_S11, 1),
        "S22": (1, 1, COL_S22, set(CSUM_TRI), CS_S22, 1),
    }

    with (
        tc.tile_pool(name="sm_sb", bufs=3) as sm,
        tc.tile_pool(name="cs_sb", bufs=2) as cs_sb,
        tc.tile_pool(name="sm_ps", bufs=2, space="PSUM") as sm_ps,
        tc.tile_pool(name="cs12_ps", bufs=1, space="PSUM") as cs12_ps,
        tc.tile_pool(name="cstri_ps", bufs=1, space="PSUM") as cstri_ps,
    ):
        csum_live = {}     # (sim) -> (cs tiles, c)
        pending = []       # deferred csum matmuls: one-unit PE pipeline

        def flush_pending():
            while pending:
                fn = pending.pop(0)
                fn()

        for ui, (sim, c, b) in enumerate(units):
            lh, rh, col0, csum_chunks, cs0, cs_pool_id = SIM_CFG[sim]
            is12 = sim == "S12"
            if is12:
                m_tiles = masks_for.pop((sim, c, b))
                nxt = s12_next.get((sim, c, b))
                if nxt is not None:
                    masks_for[nxt] = mask_tiles(nxt[1], nxt[2])

            lc = lh * N + b * 128
            rcs = rh * N + c * CCH
            s_ps = sm_ps.tile([128, CCH], F32, name="s_ps", tag="s")
            for k in range(4):
                for n in range(2):
                    ns = n * 512
                    nc.tensor.matmul(
                        s_ps[:, ns:ns + 512],
                        ht[k][:, lc:lc + 128],
                        ht[k][:, rcs + ns:rcs + ns + 512],
                        start=(k == 0), stop=(k == 3))

            # previous unit's colsum matmuls go behind this unit's sims on PE
            flush_pending()

            e_sb = sm.tile([128, CCH], BF16, name="e_sb", tag="e")
            col = col0 + c * NBLK + b
            nc.scalar.activation(
                e_sb[:], s_ps[:], AF.Exp, scale=2.0,
                accum_out=acc_all[:, col:col + 1])

            if is12:
                for mi, m_t in enumerate(m_tiles):
                    mcol = (COL_P12, COL_N12, COL_P21, COL_N21)[mi] + c * NBLK + b
                    tsc = sm.tile([128, CCH], BF16, name="tsc",
                                  tag="tsc", bufs=1)
                    nc.vector.scalar_tensor_tensor(
                        tsc[:], e_sb[:], 1.0, m_t[:],
                        op0=OP.mult, op1=OP.mult,
                        accum_out=acc_all[:, mcol:mcol + 1])

            if c in csum_chunks:
                cpool = cs12_ps if cs_pool_id == 0 else cstri_ps
                if sim not in csum_live:
                    csum_live[sim] = (
                        [cpool.tile([1, 512], F32, name=f"cs{sim}{h}",
                                    tag=f"cs{cs_pool_id}{h}") for h in range(2)],
                        c)
                cs_tiles, cc = csum_live[sim]
                assert cc == c, (sim, cc, c)

                def emit_csum(cs_tiles=cs_tiles, e_sb=e_sb, b=b, sim=sim,
                              c=c, cs0=cs0, csum_chunks=csum_chunks):
                    for h in range(2):
                        nc.tensor.matmul(
                            cs_tiles[h][:], ones_cb[:],
                            e_sb[:, h * 512:(h + 1) * 512],
                            start=(b == 0), stop=(b == NBLK - 1))
                    if b == NBLK - 1:
                        # drain colsums: PSUM -> SBUF staging -> DRAM
                        stage = cs_sb.tile([1, CCH], F32, name="cstage",
                                           tag="cstage")
                        for h in range(2):
                            nc.vector.tensor_copy(
                                stage[:, h * 512:(h + 1) * 512], cs_tiles[h][:])
                        if sim == "S12":
                            off = cs0 + c * CCH
                        else:
                            off = cs0 + (c - 1) * CCH
                        nc.sync.dma_start(
                            csum_out[:, off:off + CCH], stage[:])
                        del csum_live[sim]

                pending.append(emit_csum)

        flush_pending()
        nc.sync.dma_start(acc_out[:], acc_all[:])
    mk.release()
    top.release()


_CACHE = {}


def _build(reps=1):
    key = ("nc", reps)
    if key in _CACHE:
        return _CACHE[key]
    _patch_act_tables()
    nc = bacc.Bacc("TRN2", target_bir_lowering=False, debug=False,
                   enable_asserts=False, num_devices=NCORES)
    with tile.TileContext(nc) as tc:
        _emit(nc, tc, reps=reps)
    nc.compile()
    _CACHE[key] = nc
    return nc


def prepare_in_maps(z1, z2, pos_mask, neg_mask, W1, b1, W2, b2):
    bf16 = ml_dtypes.bfloat16
    w1t_aug = np.concatenate([W1.T, b1[None, :]], axis=0).astype(bf16)
    w2t = np.ascontiguousarray(W2.T).astype(bf16)
    b2t = np.ascontiguousarray(b2[:, None]).astype(np.float32)
    pm_b = np.asarray(pos_mask).astype(bf16)
    nm_b = np.asarray(neg_mask).astype(bf16)
    pmt_b = np.ascontiguousarray(pm_b.T)
    nmt_b = np.ascontiguousarray(nm_b.T)

    in_maps = []
    for d in range(NCORES):
        r0 = d * RPC
        z1r = np.roll(z1, -r0, axis=0)
        z2r = np.roll(z2, -r0, axis=0)
        zt = np.ascontiguousarray(
            np.concatenate([z1r, z2r], axis=0).T).astype(bf16)
        in_maps.append({
            "zt": zt, "w1t": w1t_aug, "w2t": w2t, "b2t": b2t,
            "pm": np.ascontiguousarray(np.roll(pm_b[r0:r0 + RPC], -r0, axis=1)),
            "nm": np.ascontiguousarray(np.roll(nm_b[r0:r0 + RPC], -r0, axis=1)),
            "pmt": np.ascontiguousarray(np.roll(pmt_b[r0:r0 + RPC], -r0, axis=1)),
            "nmt": np.ascontiguousarray(np.roll(nmt_b[r0:r0 + RPC], -r0, axis=1)),
        })
    return in_maps


def finalize(results):
    """Host tail math (f64): per-core acc/csum/d12p -> (unsup, semi)."""
    e2 = np.exp(2.0)

    def rowsums(acc, col0, nchunks):
        # acc cols col0 + c*8 + b; rows for (b, p) -> local row b*128+p
        cols = acc[:, col0:col0 + nchunks * NBLK]         # [128, nc*8]
        cols = cols.reshape(128, nchunks, NBLK).sum(1)    # [128, 8]
        return cols.T.reshape(RPC)                        # local row order

    accs = [r["acc"].astype(np.float64) for r in results]
    csums = [r["csum"][0].astype(np.float64) for r in results]

    rs11 = np.zeros(N)
    rs22 = np.zeros(N)
    rs12 = np.zeros(N)
    rs21 = np.zeros(N)
    mk = np.zeros(4)                  # p12, n12, p21, n21
    d12 = np.zeros(N)
    for d in range(NCORES):
        r0 = d * RPC
        acc = accs[d]
        rs12[r0:r0 + RPC] = rowsums(acc, COL_S12, NC12)
        rs11[r0:r0 + RPC] = rowsums(acc, COL_S11, NCTRI)
        rs22[r0:r0 + RPC] = rowsums(acc, COL_S22, NCTRI)
        for mi, col0 in enumerate((COL_P12, COL_N12, COL_P21, COL_N21)):
            mk[mi] += acc[:, col0:col0 + NC12 * NBLK].sum()
        d12[r0:r0 + RPC] = results[d]["d12p"][0].astype(np.float64)
        # E12 col sums -> rs21 (rolled cols c of core d = global col r0+c)
        cs = csums[d][CS_S12:CS_S12 + N]
        rs21 += np.roll(cs, r0)
    # triangle transpose completion for S11/S22: ordered pair (a, a+delta),
    # delta in {5,6,7}, comes from core b=(a+delta)%8, chunk k=8-delta.
    for d in range(NCORES):
        for k in CSUM_TRI:            # chunk k covers global block (d+k)%8
            a = (d + k) % NCORES
            delta = (NCORES - k)      # in {5,6,7}; pair (a, a+delta) == (a, d)
            assert (a + delta) % NCORES == d
            rs11[a * RPC:(a + 1) * RPC] += csums[d][
                CS_S11 + (k - 1) * RPC:CS_S11 + k * RPC]
            rs22[a * RPC:(a + 1) * RPC] += csums[d][
                CS_S22 + (k - 1) * RPC:CS_S22 + k * RPC]

    num = np.exp(2.0 * d12)
    l1 = -np.log(num / (rs11 + rs12 - e2))
    l2 = -np.log(num / (rs22 + rs21 - e2))
    unsup = 0.5 * (l1 + l2).sum() / N

    tr = num.sum()
    p12, n12, p21, n21 = mk
    s1 = -np.log(p12 / (p12 + (n12 - tr)))
    s2 = -np.log(p21 / (p21 + (n21 - tr)))
    semi = 0.5 * (s1 + s2)

    return (np.float32(unsup), np.float32(semi))


def kernel(z1, z2, pos_mask, neg_mask, W1, b1, W2, b2):
    nc = _build()
    in_maps = prepare_in_maps(z1, z2, pos_mask, neg_mask, W1, b1, W2, b2)
    res = run_bass_kernel_spmd(nc, in_maps, core_ids=list(range(NCORES)))
    return finalize(res.results)


# revision 14
# speedup vs baseline: 129944.9373x; 129944.9373x over previous
"""Trainium2 Bass kernel for nn_Loss_2482491097912 (SimCLR-style semi-supervised loss).

Strategy (8 NeuronCores, data-parallel over anchor rows):
  - Each core receives the FULL z1/z2 (pre-transposed + bf16 on host) and its
    1024-row slice of the masks, with h-columns ROLLED so that every core's
    local rows sit at columns [0:1024] -> one shared SPMD program.
  - On-core: projection MLP in transposed layout (HT = [feat, rows]),
    column-normalize (1/||h|| via Ln/Exp), then sim matmuls + exp with fused
    row-sum accumulation, masked sums via DVE tensor ops.
  - v2 improvements over the first working version:
      * S21 is never computed: its row sums are column sums of E12 (PSUM-
        accumulated ones-matmuls) and its masked sums use host-transposed
        masks applied to E12.
      * S11/S22 exploit symmetry: only 5 of 8 rolled column blocks are
        computed per core (cyclic triangle cover); the missing row-sum
        pieces come from column sums of blocks 1..3.
      * exp output and masked-sum operands are bf16 (2x DVE throughput);
        masks are shipped as bf16 from host.
      * activation-table steering: all scalar-engine functions resolve to
        the one table set that holds Exp+Ln+Square, removing ~65 table
        loads (~2.7us each) from the critical path.
  - Host combines per-core row sums / col sums / masked sums / diag dots
    into the two scalar losses (cheap numpy tail math).
"""

import numpy as np
import ml_dtypes

import concourse.bass as bass
import concourse.bacc as bacc
import concourse.tile as tile
import concourse.mybir as mybir
from concourse.bass_utils import run_bass_kernel_spmd

F32 = mybir.dt.float32
BF16 = mybir.dt.bfloat16
FP8 = mybir.dt.float8e4
AF = mybir.ActivationFunctionType
OP = mybir.AluOpType
DR = mybir.MatmulPerfMode.DoubleRow

# normalized H is stored as fp8e4m3 scaled by HSCALE (entries ~N(0, 0.7));
# sim matmuls then carry HSCALE^2, removed in the exp input scale.
HSCALE = 16.0
LN_HSCALE = float(np.log(HSCALE))
EXP_SCALE = 2.0 / (HSCALE * HSCALE)

N = 8192
D = 512
NCORES = 8
RPC = N // NCORES          # rows per core = 1024
NBLK = RPC // 128          # row blocks per core = 8
NH = 2 * N                 # 16384 stacked rows (h1 then h2)
PCH = 512                  # projection column chunk
NPCH = NH // PCH           # 32 projection chunks
CCH = 1024                 # phase-2 sim column chunk
NC12 = N // CCH            # 8 column chunks for S12
NCTRI = 5                  # 5 rolled column blocks for S11/S22 (triangle)
CSUM_TRI = (1, 2, 3)       # triangle chunks whose col sums we need

# acc_all rowsum/masked-sum column map (one f32 col per (sim, chunk, block)).
COL_S12 = 0                            # 8c x 8b = 64
COL_S11 = 64                           # 5c x 8b = 40
COL_S22 = 104                          # 40
COL_P12 = 144                          # 64  (E12 * pos)
COL_N12 = 208                          # 64  (E12 * neg)
COL_P21 = 272                          # 64  (E12 * pos^T)
COL_N21 = 336                          # 64  (E12 * neg^T)
ACC_COLS = 400

# csum output layout: [1, 14336] f32
CS_S12 = 0                 # 8192: colsums of E12 (rolled cols)
CS_S11 = N                 # 3072: colsums of E11 rolled chunks 1..3
CS_S22 = N + 3072          # 3072
CSUM_COLS = N + 6144


def _patch_act_tables():
    """Steer the act-table-load pass to the one set that holds Exp+Ln+Square
    (+Identity/Copy) so phase 1 never switches table sets."""
    import functools
    import concourse.hw_specs as hw_specs

    if getattr(bacc.get_activation_tables, "_steered", False):
        return
    orig = hw_specs.get_activation_tables
    keep_funcs = {AF.Exp, AF.Ln, AF.Square, AF.Identity, AF.Copy}

    @functools.cache
    def steered(arch):
        tabs = orig(arch)
        best = None
        for name, s in tabs.items():
            if keep_funcs <= s:
                best = name
                break
        if best is None:
            return tabs
        return {
            name: (s if name == best else (s - keep_funcs))
            for name, s in tabs.items()
        }

    steered._steered = True
    bacc.get_activation_tables = steered
    hw_specs.get_activation_tables = steered


def _emit(nc, tc, reps=1):
    zt = nc.dram_tensor("zt", [D, NH], BF16, kind="ExternalInput").ap()
    w1t = nc.dram_tensor("w1t", [D + 1, D], BF16, kind="ExternalInput").ap()
    w2t = nc.dram_tensor("w2t", [D, D], BF16, kind="ExternalInput").ap()
    b2t = nc.dram_tensor("b2t", [D, 1], F32, kind="ExternalInput").ap()
    pm = nc.dram_tensor("pm", [RPC, N], BF16, kind="ExternalInput").ap()
    nm = nc.dram_tensor("nm", [RPC, N], BF16, kind="ExternalInput").ap()
    pmt = nc.dram_tensor("pmt", [RPC, N], BF16, kind="ExternalInput").ap()
    nmt = nc.dram_tensor("nmt", [RPC, N], BF16, kind="ExternalInput").ap()

    acc_out = nc.dram_tensor("acc", [128, ACC_COLS], F32, kind="ExternalOutput").ap()
    csum_out = nc.dram_tensor("csum", [1, CSUM_COLS], F32, kind="ExternalOutput").ap()
    d12_out = nc.dram_tensor("d12p", [1, RPC], F32, kind="ExternalOutput").ap()
    for _ in range(reps):
        _emit_body(nc, tc, zt, w1t, w2t, b2t, pm, nm, pmt, nmt,
                   acc_out, csum_out, d12_out)


def _emit_body(nc, tc, zt, w1t, w2t, b2t, pm, nm, pmt, nmt,
               acc_out, csum_out, d12_out):

    zt_r = zt.rearrange("(t p) n -> p t n", p=128)      # [128, 4, 16384]
    w1t_r = w1t[0:D, :].rearrange("(t p) m -> p t m", p=128)
    w2t_r = w2t.rearrange("(t p) m -> p t m", p=128)
    b2t_r = b2t.rearrange("(t p) 1 -> p t", p=128)      # [128, 4]

    top = tc.alloc_tile_pool(name="top", bufs=1)
    # resident normalized H^T in fp8 (x HSCALE), DoubleRow-interleaved:
    # hp[j][:, i, col] holds k-tile (2j+i); [128, 2, 16384] = 32KB/part each
    hp = [top.tile([128, 2, NH], FP8, name=f"hp{j}", tag=f"hp{j}")
          for j in range(2)]
    acc_all = top.tile([128, ACC_COLS], F32, name="acc_all")
    w1_sb = top.tile([128, 4, D], BF16, name="w1_sb")
    b1_sb = top.tile([1, D], BF16, name="b1_sb")
    w2_sb = top.tile([128, 4, D], BF16, name="w2_sb")
    b2_sb = top.tile([128, 4], F32, name="b2_sb")
    ones_r = top.tile([1, D], BF16, name="ones_r")      # rhs for L1 bias matmul
    ones_cb = top.tile([128, 1], BF16, name="ones_cb")  # bf16 lhsT for col sums
    lnh_sb = top.tile([1, 1], F32, name="lnh_sb")       # ln(HSCALE) bias

    nc.sync.dma_start(w1_sb[:], w1t_r)
    nc.sync.dma_start(b1_sb[:], w1t[D:D + 1, :])
    nc.sync.dma_start(w2_sb[:], w2t_r)
    nc.sync.dma_start(b2_sb[:], b2t_r)
    nc.vector.memset(ones_r[:], 1.0)
    nc.vector.memset(lnh_sb[:], LN_HSCALE)
    nc.vector.memset(ones_cb[:], 1.0)
    nc.vector.memset(acc_all[:], 0.0)

    # ---------------- Phase 1: projection (transposed layout) ----------------
    with (
        tc.tile_pool(name="pj_sb", bufs=2) as pj,
        tc.tile_pool(name="pp_l1", bufs=2, space="PSUM") as pp_l1,
        tc.tile_pool(name="pp_l2", bufs=1, space="PSUM") as pp_l2,
        tc.tile_pool(name="pp_n", bufs=2, space="PSUM") as pp_n,
    ):
        for c in range(NPCH):
            cs = c * PCH
            zt_t = pj.tile([128, 4, PCH], BF16, name="zt_t", tag="zt")
            nc.sync.dma_start(zt_t[:], zt_r[:, :, cs:cs + PCH])

            gts = []
            for m in range(4):
                ms = m * 128
                l1_ps = pp_l1.tile([128, PCH], F32, name="l1_ps", tag="l1")
                for k in range(4):
                    nc.tensor.matmul(
                        l1_ps[:], w1_sb[:, k, ms:ms + 128], zt_t[:, k, :],
                        start=(k == 0), stop=False)
                # bias row via K=1 matmul: adds b1[m-chunk] to all columns
                nc.tensor.matmul(
                    l1_ps[:], b1_sb[:, ms:ms + 128], ones_r[:, 0:PCH],
                    start=False, stop=True)
                t_sb = pj.tile([128, PCH], F32, name="t_sb", tag="texp")
                nc.scalar.activation(t_sb[:], l1_ps[:], AF.Exp)
                gt = pj.tile([128, PCH], BF16, name="gt", tag=f"gt{m}")
                # elu(u) = min(exp(u) - 1, u)
                nc.vector.scalar_tensor_tensor(
                    gt[:], t_sb[:], 1.0, l1_ps[:], op0=OP.subtract, op1=OP.min)
                gts.append(gt)

            norms_ps = pp_n.tile([1, PCH], F32, name="norms_ps", tag="n")
            l2s = []
            for m in range(4):
                ms = m * 128
                l2_ps = pp_l2.tile([128, PCH], F32, name="l2_ps", tag=f"l2{m}")
                for k in range(4):
                    nc.tensor.matmul(
                        l2_ps[:], w2_sb[:, k, ms:ms + 128], gts[k][:],
                        start=(k == 0), stop=(k == 3))
                sq_sb = pj.tile([128, PCH], BF16, name="sq_sb", tag="sq")
                nc.scalar.activation(
                    sq_sb[:], l2_ps[:], AF.Square, bias=b2_sb[:, m:m + 1])
                nc.tensor.matmul(
                    norms_ps[:], ones_cb[:], sq_sb[:],
                    start=(m == 0), stop=(m == 3))
                l2s.append(l2_ps)

            # r = HSCALE/sqrt(norms^2) = exp(-0.5 * ln(norms^2) + ln HSCALE)
            lg = pj.tile([1, PCH], F32, name="lg", tag="lg")
            nc.scalar.activation(lg[:], norms_ps[:], AF.Ln)
            r1 = pj.tile([1, PCH], F32, name="r1", tag="r1")
            nc.scalar.activation(r1[:], lg[:], AF.Exp, scale=-0.5,
                                 bias=lnh_sb[:])
            rb = pj.tile([128, PCH], F32, name="rb", tag="rb")
            nc.gpsimd.partition_broadcast(rb[:], r1[:])

            for m in range(4):
                # htn = (h + b2) * r -> fp8 into resident HP (DR interleave)
                nc.vector.scalar_tensor_tensor(
                    hp[m // 2][:, m % 2, cs:cs + PCH], l2s[m][:],
                    b2_sb[:, m:m + 1], rb[:],
                    op0=OP.add, op1=OP.mult)

    # ---------------- Phase 2 pools (masks prefetch before d12) --------------
    mk = tc.alloc_tile_pool(name="mk_sb", bufs=2)
    MASK_SRCS = (pm, nm, pmt, nmt)

    def mask_tiles(c, b):
        tiles = []
        for mi, src in enumerate(MASK_SRCS):
            t = mk.tile([128, CCH], BF16, name=f"m{mi}", tag=f"m{mi}")
            nc.sync.dma_start(
                t[:], src[b * 128:(b + 1) * 128, c * CCH:(c + 1) * CCH])
            tiles.append(t)
        return tiles

    # Build the interleaved unit schedule: S12 units carry the DVE load
    # (4 masked stt each), triangle units are PE-only; interleave so both
    # engines stay busy.
    s12_units = [("S12", c, b) for c in range(NC12) for b in range(NBLK)]
    tri_units = ([("S11", c, b) for c in range(NCTRI) for b in range(NBLK)] +
                 [("S22", c, b) for c in range(NCTRI) for b in range(NBLK)])
    tagged = ([((i + 0.5) / len(s12_units), u) for i, u in enumerate(s12_units)] +
              [((i + 0.5) / len(tri_units), u) for i, u in enumerate(tri_units)])
    tagged.sort(key=lambda t: t[0])
    units = [u for _, u in tagged]

    # masks are prefetched one full S12 unit ahead; seed the pipeline here so
    # the first DMA overlaps the d12 phase
    s12_next = {s12_units[i]: s12_units[i + 1]
                for i in range(len(s12_units) - 1)}
    masks_for = {s12_units[0]: mask_tiles(s12_units[0][1], s12_units[0][2])}

    # ------------- Phase 1.5: d12 = rowwise dot n1.n2 for local rows ---------
    with (
        tc.tile_pool(name="dd_sb", bufs=2) as dd,
        tc.tile_pool(name="dd_ps", bufs=2, space="PSUM") as dd_ps,
    ):
        d12_sb = dd.tile([1, RPC], F32, name="d12_sb", bufs=1)
        for h in range(2):
            hs = h * 512
            dps = dd_ps.tile([1, 512], F32, name="dps", tag="dps")
            for k in range(4):
                j, i = k // 2, k % 2
                mt = dd.tile([128, 512], BF16, name="mt", tag="mt")
                nc.vector.tensor_mul(
                    mt[:], hp[j][:, i, hs:hs + 512],
                    hp[j][:, i, N + hs:N + hs + 512])
                nc.tensor.matmul(dps[:], ones_cb[:], mt[:],
                                 start=(k == 0), stop=(k == 3))
            # values carry HSCALE^2; host divides it back out
            nc.scalar.copy(d12_sb[:, hs:hs + 512], dps[:])
        nc.sync.dma_start(d12_out[:], d12_sb[:])

    # ---------------- Phase 2: sims + exp row-sums + masked/col sums ---------
    SIM_CFG = {
        # lhs half, rhs half, rowsum col base, csum chunks, csum col base
        "S12": (0, 1, COL_S12, set(range(NC12)), CS_S12, 0),
        "S11": (0, 0, COL_S11, set(CSUM_TRI), CS_S11, 1),
        "S22": (1, 1, COL_S22, set(CSUM_TRI), CS_S22, 1),
    }

    with (
        tc.tile_pool(name="sm_sb", bufs=3) as sm,
        tc.tile_pool(name="cs_sb", bufs=2) as cs_sb,
        tc.tile_pool(name="sm_ps", bufs=2, space="PSUM") as sm_ps,
        tc.tile_pool(name="cs12_ps", bufs=1, space="PSUM") as cs12_ps,
        tc.tile_pool(name="cstri_ps", bufs=1, space="PSUM") as cstri_ps,
    ):
        csum_live = {}     # (sim) -> (cs tiles, c)
        pending = []       # deferred csum matmuls: one-unit PE pipeline

        def flush_pending():
            while pending:
                fn = pending.pop(0)
                fn()

        for ui, (sim, c, b) in enumerate(units):
            lh, rh, col0, csum_chunks, cs0, cs_pool_id = SIM_CFG[sim]
            is12 = sim == "S12"
            if is12:
                m_tiles = masks_for.pop((sim, c, b))
                nxt = s12_next.get((sim, c, b))
                if nxt is not None:
                    masks_for[nxt] = mask_tiles(nxt[1], nxt[2])

            lc = lh * N + b * 128
            rcs = rh * N + c * CCH
            s_ps = sm_ps.tile([128, CCH], F32, name="s_ps", tag="s")
            for n in range(2):
                ns = n * 512
                for j in range(2):
                    nc.tensor.matmul(
                        s_ps[:, ns:ns + 512],
                        hp[j][:, :, lc:lc + 128],
                        hp[j][:, :, rcs + ns:rcs + ns + 512],
                        start=(j == 0), stop=(j == 1), perf_mode=DR)

            # previous unit's colsum matmuls go behind this unit's sims on PE
            flush_pending()

            e_sb = sm.tile([128, CCH], BF16, name="e_sb", tag="e")
            col = col0 + c * NBLK + b
            nc.scalar.activation(
                e_sb[:], s_ps[:], AF.Exp, scale=EXP_SCALE,
                accum_out=acc_all[:, col:col + 1])

            if is12:
                for mi, m_t in enumerate(m_tiles):
                    mcol = (COL_P12, COL_N12, COL_P21, COL_N21)[mi] + c * NBLK + b
                    eng = nc.vector if mi < 2 else nc.gpsimd
                    tsc = sm.tile([128, CCH], BF16, name="tsc",
                                  tag=f"tsc{mi}", bufs=1)
                    eng.scalar_tensor_tensor(
                        tsc[:], e_sb[:], 1.0, m_t[:],
                        op0=OP.mult, op1=OP.mult,
                        accum_out=acc_all[:, mcol:mcol + 1])

            if c in csum_chunks:
                cpool = cs12_ps if cs_pool_id == 0 else cstri_ps
                if sim not in csum_live:
                    csum_live[sim] = (
                        [cpool.tile([1, 512], F32, name=f"cs{sim}{h}",
                                    tag=f"cs{cs_pool_id}{h}") for h in range(2)],
                        c)
                cs_tiles, cc = csum_live[sim]
                assert cc == c, (sim, cc, c)

                def emit_csum(cs_tiles=cs_tiles, e_sb=e_sb, b=b, sim=sim,
                              c=c, cs0=cs0, csum_chunks=csum_chunks):
                    for h in range(2):
                        nc.tensor.matmul(
                            cs_tiles[h][:], ones_cb[:],
                            e_sb[:, h * 512:(h + 1) * 512],
                            start=(b == 0), stop=(b == NBLK - 1))
                    if b == NBLK - 1:
                        # drain colsums: PSUM -> SBUF staging -> DRAM
                        stage = cs_sb.tile([1, CCH], F32, name="cstage",
                                           tag="cstage")
                        for h in range(2):
                            nc.vector.tensor_copy(
                                stage[:, h * 512:(h + 1) * 512], cs_tiles[h][:])
                        if sim == "S12":
                            off = cs0 + c * CCH
                        else:
                            off = cs0 + (c - 1) * CCH
                        nc.sync.dma_start(
                            csum_out[:, off:off + CCH], stage[:])
                        del csum_live[sim]

                pending.append(emit_csum)

        flush_pending()
        nc.sync.dma_start(acc_out[:], acc_all[:])
    mk.release()
    top.release()


_CACHE = {}


def _build(reps=1):
    key = ("nc", reps)
    if key in _CACHE:
        return _CACHE[key]
    _patch_act_tables()
    nc = bacc.Bacc("TRN2", target_bir_lowering=False, debug=False,
                   enable_asserts=False, num_devices=NCORES)
    with tile.TileContext(nc) as tc:
        _emit(nc, tc, reps=reps)
    nc.compile()
    _CACHE[key] = nc
    return nc


def prepare_in_maps(z1, z2, pos_mask, neg_mask, W1, b1, W2, b2):
    bf16 = ml_dtypes.bfloat16
    w1t_aug = np.concatenate([W1.T, b1[None, :]], axis=0).astype(bf16)
    w2t = np.ascontiguousarray(W2.T).astype(bf16)
    b2t = np.ascontiguousarray(b2[:, None]).astype(np.float32)
    pm_b = np.asarray(pos_mask).astype(bf16)
    nm_b = np.asarray(neg_mask).astype(bf16)
    pmt_b = np.ascontiguousarray(pm_b.T)
    nmt_b = np.ascontiguousarray(nm_b.T)

    in_maps = []
    for d in range(NCORES):
        r0 = d * RPC
        z1r = np.roll(z1, -r0, axis=0)
        z2r = np.roll(z2, -r0, axis=0)
        zt = np.ascontiguousarray(
            np.concatenate([z1r, z2r], axis=0).T).astype(bf16)
        in_maps.append({
            "zt": zt, "w1t": w1t_aug, "w2t": w2t, "b2t": b2t,
            "pm": np.ascontiguousarray(np.roll(pm_b[r0:r0 + RPC], -r0, axis=1)),
            "nm": np.ascontiguousarray(np.roll(nm_b[r0:r0 + RPC], -r0, axis=1)),
            "pmt": np.ascontiguousarray(np.roll(pmt_b[r0:r0 + RPC], -r0, axis=1)),
            "nmt": np.ascontiguousarray(np.roll(nmt_b[r0:r0 + RPC], -r0, axis=1)),
        })
    return in_maps


def finalize(results):
    """Host tail math (f64): per-core acc/csum/d12p -> (unsup, semi)."""
    e2 = np.exp(2.0)

    def rowsums(acc, col0, nchunks):
        # acc cols col0 + c*8 + b; rows for (b, p) -> local row b*128+p
        cols = acc[:, col0:col0 + nchunks * NBLK]         # [128, nc*8]
        cols = cols.reshape(128, nchunks, NBLK).sum(1)    # [128, 8]
        return cols.T.reshape(RPC)                        # local row order

    accs = [r["acc"].astype(np.float64) for r in results]
    csums = [r["csum"][0].astype(np.float64) for r in results]

    rs11 = np.zeros(N)
    rs22 = np.zeros(N)
    rs12 = np.zeros(N)
    rs21 = np.zeros(N)
    mk = np.zeros(4)                  # p12, n12, p21, n21
    d12 = np.zeros(N)
    for d in range(NCORES):
        r0 = d * RPC
        acc = accs[d]
        rs12[r0:r0 + RPC] = rowsums(acc, COL_S12, NC12)
        rs11[r0:r0 + RPC] = rowsums(acc, COL_S11, NCTRI)
        rs22[r0:r0 + RPC] = rowsums(acc, COL_S22, NCTRI)
        for mi, col0 in enumerate((COL_P12, COL_N12, COL_P21, COL_N21)):
            mk[mi] += acc[:, col0:col0 + NC12 * NBLK].sum()
        d12[r0:r0 + RPC] = (results[d]["d12p"][0].astype(np.float64)
                            / (HSCALE * HSCALE))
        # E12 col sums -> rs21 (rolled cols c of core d = global col r0+c)
        cs = csums[d][CS_S12:CS_S12 + N]
        rs21 += np.roll(cs, r0)
    # triangle transpose completion for S11/S22: ordered pair (a, a+delta),
    # delta in {5,6,7}, comes from core b=(a+delta)%8, chunk k=8-delta.
    for d in range(NCORES):
        for k in CSUM_TRI:            # chunk k covers global block (d+k)%8
            a = (d + k) % NCORES
            delta = (NCORES - k)      # in {5,6,7}; pair (a, a+delta) == (a, d)
            assert (a + delta) % NCORES == d
            rs11[a * RPC:(a + 1) * RPC] += csums[d][
                CS_S11 + (k - 1) * RPC:CS_S11 + k * RPC]
            rs22[a * RPC:(a + 1) * RPC] += csums[d][
                CS_S22 + (k - 1) * RPC:CS_S22 + k * RPC]

    num = np.exp(2.0 * d12)
    l1 = -np.log(num / (rs11 + rs12 - e2))
    l2 = -np.log(num / (rs22 + rs21 - e2))
    unsup = 0.5 * (l1 + l2).sum() / N

    tr = num.sum()
    p12, n12, p21, n21 = mk
    s1 = -np.log(p12 / (p12 + (n12 - tr)))
    s2 = -np.log(p21 / (p21 + (n21 - tr)))
    semi = 0.5 * (s1 + s2)

    return (np.float32(unsup), np.float32(semi))


def kernel(z1, z2, pos_mask, neg_mask, W1, b1, W2, b2):
    nc = _build()
    in_maps = prepare_in_maps(z1, z2, pos_mask, neg_mask, W1, b1, W2, b2)
    res = run_bass_kernel_spmd(nc, in_maps, core_ids=list(range(NCORES)))
    return finalize(res.results)
